# revision 18
# baseline (speedup 1.0000x reference)
"""Conv2D (N=32, Cin=128, 56x56 -> Cout=256, 3x3, pad 1, stride 1) on 8 Trainium2
NeuronCores.

Strategy: data-parallel over batch (4 images per core), conv lowered to 9
shifted matmuls (one per filter tap) accumulating in PSUM over the
Cin=128-partition contraction dim.  Cout=256 is handled as 2 halves of 128
output partitions.

The input is zero-padded on the host to 59x58 (1 top / 2 bottom pad rows, 1
left / 1 right pad col) and flattened per image to a contiguous 3422-element
stream per cin partition.  A tap (kh, kw) read is then a single CONTIGUOUS
464-element span starting at kh*58+kw within an 11-row strip: the matmul
computes an 8-row x 58-col block of the *padded* output grid, and the two
garbage columns (j=56,57) are simply not written back to DRAM.  Contiguous
moving APs keep the PE streaming at 1 column/cycle with no per-row AP-walk
overhead.

Input is DMA'd in 11-row strips (one per 8-row output block, 2-row halo) so
the first matmul only waits for the first 320 KB strip, not the whole image.
DMA rings: strips on nc.sync (HWDGE ring 0), weights/bias on nc.scalar
(HWDGE ring 1), outputs on nc.gpsimd (SWDGE) - three independent FIFOs.

Matmul dtype fp32r: the PE's single-pass fp32 mode (8-bit exp / 11-bit
mantissa, top 20 bits of the fp32 word) streaming at 1 column/cycle - 4x
faster than exact fp32 (4 cyc/row).  Inputs are pre-rounded to the fp32r
grid on the host so hardware truncation lands on round-to-nearest values;
measured scale-rel error vs the fp32 reference is ~1.5e-4.
"""

import os
import sys

import numpy as np

sys.path.insert(0, "/opt/trn_rl_repo")

import concourse.tile as tile
from concourse import bacc, mybir

N, CIN, H, W = 32, 128, 56, 56
COUT, KH, KW = 256, 3, 3
NCORES = 8
NPER = N // NCORES  # images per core
HP, WP = H + 3, W + 2  # padded spatial dims (1 top + 2 bottom, 1 left + 1 right)
FLAT = HP * WP  # 3422 padded pixels per image per cin
RB = 8  # output rows per PSUM chunk
NRB = H // RB  # 7 row-blocks per image
CHUNK = RB * WP  # 464 <= 512 fp32 PSUM bank limit
STRIP_ROWS = RB + KH  # 11 padded rows per input strip (8 + 2 halo + 1 overread)
STRIP = STRIP_ROWS * WP  # 638
NTAP = KH * KW

MM_MODE = os.environ.get("CONV_MM_MODE", "fp32r")

_CACHE = {}


def _build(mm_mode):
    f32 = mybir.dt.float32
    in_dt = {
        "fp32": f32,
        "fp32r": mybir.dt.float32r,
        "bf16": mybir.dt.bfloat16,
    }[mm_mode]

    nc = bacc.Bacc(None, target_bir_lowering=False)
    xp_d = nc.declare_dram_parameter("xp", [NPER, CIN, FLAT], in_dt, isOutput=False)
    w_d = nc.declare_dram_parameter("w", [CIN, NTAP, COUT], in_dt, isOutput=False)
    b_d = nc.declare_dram_parameter("b", [CIN, 2], f32, isOutput=False)
    y_d = nc.declare_dram_parameter("y", [NPER, COUT, H, W], f32, isOutput=True)

    with tile.TileContext(nc) as tc:
        with (
            tc.tile_pool(name="xin", bufs=12) as xpool,
            tc.tile_pool(name="wgt", bufs=1) as wpool,
            tc.tile_pool(name="bias", bufs=1) as bpool,
            tc.tile_pool(name="out", bufs=8) as opool,
            tc.tile_pool(name="ps", bufs=8, space="PSUM") as pspool,
        ):
            # PE warm-up: HAM un-throttles (K=4/8 -> 8/8, 1.2 -> 2.4 GHz) only
            # after ~3.4us of sustained PE activity.  Burn the initial
            # DMA-wait window on dummy matmuls over a memset tile so the real
            # matmuls start at full clock.
            wrm = bpool.tile([CIN, CHUNK], mybir.dt.bfloat16, tag="warm")
            nc.vector.memset(wrm[:], 0)
            wps = pspool.tile([128, CHUNK], f32, tag="ps")
            for _ in range(16):
                nc.tensor.matmul(wps[:], wrm[:, 0:128], wrm[:], start=True, stop=True)

            # one tile per tap so an MM only waits on its own tap's DMA
            w_taps = []
            for tap in range(NTAP):
                wt = wpool.tile([CIN, COUT], in_dt, tag=f"w{tap}")
                nc.scalar.dma_start(out=wt[:], in_=w_d[:, tap, :])
                w_taps.append(wt)
            b_sb = bpool.tile([CIN, 2], f32)
            nc.scalar.dma_start(out=b_sb[:], in_=b_d[:, :])

            for i in range(NPER):
                for rb in range(NRB):
                    xt = xpool.tile([CIN, STRIP], in_dt, tag="xs")
                    r0 = rb * RB
                    nc.sync.dma_start(
                        out=xt[:], in_=xp_d[i, :, r0 * WP : r0 * WP + STRIP]
                    )
                    for half in range(2):
                        ps = pspool.tile([128, RB, WP], f32)
                        for tap in range(NTAP):
                            kh, kw = divmod(tap, KW)
                            off = kh * WP + kw
                            nc.tensor.matmul(
                                ps[:],
                                w_taps[tap][:, half * 128 : half * 128 + 128],
                                xt[:, off : off + CHUNK],
                                start=(tap == 0),
                                stop=(tap == NTAP - 1),
                            )
                        ot = opool.tile([128, RB, WP], f32)
                        nc.vector.tensor_scalar_add(
                            ot[:], ps[:], b_sb[:, half : half + 1]
                        )
                        nc.scalar.dma_start(
                            out=y_d[i, half * 128 : half * 128 + 128, r0 : r0 + RB, :],
                            in_=ot[:, :, 0:W],
                        )
    nc.finalize()
    return nc


def get_nc(mm_mode=None):
    mm_mode = mm_mode or MM_MODE
    if mm_mode not in _CACHE:
        _CACHE[mm_mode] = _build(mm_mode)
    return _CACHE[mm_mode]


def _round_fp32r(a):
    """Round fp32 array to the fp32r grid (8-bit exp, 11-bit mantissa, top 20
    bits of the word) with round-to-nearest so the PE's truncation of the low
    12 bits lands on the nearest representable value."""
    u = np.ascontiguousarray(a, np.float32).view(np.uint32)
    u = u + 0x7FF + ((u >> 12) & 1)
    u &= np.uint32(0xFFFFF000)
    return u.view(np.float32)


def prep_inputs(x, weight, bias, mm_mode=None):
    """Host-side staging: zero-pad x to 59x58 and flatten, retile weights to
    [cin, tap, cout], split per-core input maps."""
    mm_mode = mm_mode or MM_MODE
    x = np.asarray(x, np.float32)
    weight = np.asarray(weight, np.float32)
    bias = np.asarray(bias, np.float32)

    xp = np.zeros((N, CIN, HP, WP), np.float32)
    xp[:, :, 1 : H + 1, 1 : W + 1] = x
    # [cout, cin, kh, kw] -> [cin, tap, cout]
    w_prep = np.ascontiguousarray(weight.transpose(1, 2, 3, 0).reshape(CIN, NTAP, COUT))
    if mm_mode == "bf16":
        import ml_dtypes

        xp = xp.astype(ml_dtypes.bfloat16)
        w_prep = w_prep.astype(ml_dtypes.bfloat16)
    elif mm_mode == "fp32r":
        xp = _round_fp32r(xp)
        w_prep = _round_fp32r(w_prep)
    xp = xp.reshape(N, CIN, FLAT)
    b_prep = np.ascontiguousarray(bias.reshape(2, 128).T.astype(np.float32))

    return [
        {
            "xp": np.ascontiguousarray(xp[c * NPER : (c + 1) * NPER]),
            "w": w_prep,
            "b": b_prep,
        }
        for c in range(NCORES)
    ]


def kernel(x, weight, bias, mm_mode=None, trace=False, tmpdir=None):
    from concourse.bass_utils import run_bass_kernel_spmd

    nc = get_nc(mm_mode)
    in_maps = prep_inputs(x, weight, bias, mm_mode)
    res = run_bass_kernel_spmd(
        nc, in_maps, list(range(NCORES)), trace=trace, tmpdir=tmpdir
    )
    out = np.concatenate([r["y"] for r in res.results], axis=0)
    if trace:
        kernel.last_results = res
    return out


# revision 19
# speedup vs baseline: 1.0328x; 1.0328x over previous
"""Conv2D (N=32, Cin=128, 56x56 -> Cout=256, 3x3, pad 1, stride 1) on 8 Trainium2
NeuronCores.

Strategy: data-parallel over batch (4 images per core), conv lowered to 9
shifted matmuls (one per filter tap) accumulating in PSUM over the
Cin=128-partition contraction dim.  Cout=256 is handled as 2 halves of 128
output partitions.

The input is zero-padded on the host to 59x58 (1 top / 2 bottom pad rows, 1
left / 1 right pad col) and flattened per image to a contiguous 3422-element
stream per cin partition.  A tap (kh, kw) read is then a single CONTIGUOUS
464-element span starting at kh*58+kw within an 11-row strip: the matmul
computes an 8-row x 58-col block of the *padded* output grid, and the two
garbage columns (j=56,57) are simply not written back to DRAM.  Contiguous
moving APs keep the PE streaming at 1 column/cycle with no per-row AP-walk
overhead.

Input is DMA'd in 11-row strips (one per 8-row output block, 2-row halo) so
the first matmul only waits for the first 320 KB strip, not the whole image.
DMA rings: strips on nc.sync (HWDGE ring 0), weights/bias on nc.scalar
(HWDGE ring 1), outputs on nc.gpsimd (SWDGE) - three independent FIFOs.

Matmul dtype fp32r: the PE's single-pass fp32 mode (8-bit exp / 11-bit
mantissa, top 20 bits of the fp32 word) streaming at 1 column/cycle - 4x
faster than exact fp32 (4 cyc/row).  Inputs are pre-rounded to the fp32r
grid on the host so hardware truncation lands on round-to-nearest values;
measured scale-rel error vs the fp32 reference is ~1.5e-4.
"""

import os
import sys

import numpy as np

sys.path.insert(0, "/opt/trn_rl_repo")

import concourse.tile as tile
from concourse import bacc, mybir

N, CIN, H, W = 32, 128, 56, 56
COUT, KH, KW = 256, 3, 3
NCORES = 8
NPER = N // NCORES  # images per core
HP, WP = H + 3, W + 2  # padded spatial dims (1 top + 2 bottom, 1 left + 1 right)
FLAT = HP * WP  # 3422 padded pixels per image per cin
RB = 8  # output rows per PSUM chunk
NRB = H // RB  # 7 row-blocks per image
CHUNK = RB * WP  # 464 <= 512 fp32 PSUM bank limit
STRIP_ROWS = RB + KH  # 11 padded rows per input strip (8 + 2 halo + 1 overread)
STRIP = STRIP_ROWS * WP  # 638
NTAP = KH * KW

MM_MODE = os.environ.get("CONV_MM_MODE", "fp32r")

_CACHE = {}


def _build(mm_mode):
    f32 = mybir.dt.float32
    in_dt = {
        "fp32": f32,
        "fp32r": mybir.dt.float32r,
        "bf16": mybir.dt.bfloat16,
    }[mm_mode]

    nc = bacc.Bacc(None, target_bir_lowering=False)
    xp_d = nc.declare_dram_parameter("xp", [NPER, CIN, FLAT], in_dt, isOutput=False)
    w_d = nc.declare_dram_parameter("w", [CIN, NTAP, COUT], in_dt, isOutput=False)
    b_d = nc.declare_dram_parameter("b", [CIN, 2], f32, isOutput=False)
    y_d = nc.declare_dram_parameter("y", [NPER, COUT, H, W], f32, isOutput=True)

    with tile.TileContext(nc) as tc:
        with (
            tc.tile_pool(name="xin", bufs=12) as xpool,
            tc.tile_pool(name="wgt", bufs=1) as wpool,
            tc.tile_pool(name="bias", bufs=1) as bpool,
            tc.tile_pool(name="out", bufs=8) as opool,
            tc.tile_pool(name="ps", bufs=8, space="PSUM") as pspool,
        ):
            # one tile per tap so an MM only waits on its own tap's DMA
            w_taps = []
            for tap in range(NTAP):
                wt = wpool.tile([CIN, COUT], in_dt, tag=f"w{tap}")
                nc.scalar.dma_start(out=wt[:], in_=w_d[:, tap, :])
                w_taps.append(wt)
            b_sb = bpool.tile([CIN, 2], f32)
            nc.scalar.dma_start(out=b_sb[:], in_=b_d[:, :])

            for i in range(NPER):
                for rb in range(NRB):
                    xt = xpool.tile([CIN, STRIP], in_dt, tag="xs")
                    r0 = rb * RB
                    nc.sync.dma_start(
                        out=xt[:], in_=xp_d[i, :, r0 * WP : r0 * WP + STRIP]
                    )
                    for half in range(2):
                        ps = pspool.tile([128, RB, WP], f32)
                        for tap in range(NTAP):
                            kh, kw = divmod(tap, KW)
                            off = kh * WP + kw
                            nc.tensor.matmul(
                                ps[:],
                                w_taps[tap][:, half * 128 : half * 128 + 128],
                                xt[:, off : off + CHUNK],
                                start=(tap == 0),
                                stop=(tap == NTAP - 1),
                            )
                        ot = opool.tile([128, RB, WP], f32)
                        nc.vector.tensor_scalar_add(
                            ot[:], ps[:], b_sb[:, half : half + 1]
                        )
                        nc.scalar.dma_start(
                            out=y_d[i, half * 128 : half * 128 + 128, r0 : r0 + RB, :],
                            in_=ot[:, :, 0:W],
                        )
    nc.finalize()
    return nc


def get_nc(mm_mode=None):
    mm_mode = mm_mode or MM_MODE
    if mm_mode not in _CACHE:
        _CACHE[mm_mode] = _build(mm_mode)
    return _CACHE[mm_mode]


def _round_fp32r(a):
    """Round fp32 array to the fp32r grid (8-bit exp, 11-bit mantissa, top 20
    bits of the word) with round-to-nearest so the PE's truncation of the low
    12 bits lands on the nearest representable value."""
    u = np.ascontiguousarray(a, np.float32).view(np.uint32)
    u = u + 0x7FF + ((u >> 12) & 1)
    u &= np.uint32(0xFFFFF000)
    return u.view(np.float32)


def prep_inputs(x, weight, bias, mm_mode=None):
    """Host-side staging: zero-pad x to 59x58 and flatten, retile weights to
    [cin, tap, cout], split per-core input maps."""
    mm_mode = mm_mode or MM_MODE
    x = np.asarray(x, np.float32)
    weight = np.asarray(weight, np.float32)
    bias = np.asarray(bias, np.float32)

    xp = np.zeros((N, CIN, HP, WP), np.float32)
    xp[:, :, 1 : H + 1, 1 : W + 1] = x
    # [cout, cin, kh, kw] -> [cin, tap, cout]
    w_prep = np.ascontiguousarray(weight.transpose(1, 2, 3, 0).reshape(CIN, NTAP, COUT))
    if mm_mode == "bf16":
        import ml_dtypes

        xp = xp.astype(ml_dtypes.bfloat16)
        w_prep = w_prep.astype(ml_dtypes.bfloat16)
    elif mm_mode == "fp32r":
        xp = _round_fp32r(xp)
        w_prep = _round_fp32r(w_prep)
    xp = xp.reshape(N, CIN, FLAT)
    b_prep = np.ascontiguousarray(bias.reshape(2, 128).T.astype(np.float32))

    return [
        {
            "xp": np.ascontiguousarray(xp[c * NPER : (c + 1) * NPER]),
            "w": w_prep,
            "b": b_prep,
        }
        for c in range(NCORES)
    ]


def kernel(x, weight, bias, mm_mode=None, trace=False, tmpdir=None):
    from concourse.bass_utils import run_bass_kernel_spmd

    nc = get_nc(mm_mode)
    in_maps = prep_inputs(x, weight, bias, mm_mode)
    res = run_bass_kernel_spmd(
        nc, in_maps, list(range(NCORES)), trace=trace, tmpdir=tmpdir
    )
    out = np.concatenate([r["y"] for r in res.results], axis=0)
    if trace:
        kernel.last_results = res
    return out
